# revision 13
# baseline (speedup 1.0000x reference)
"""MiniBert (embed + LayerNorm + single-head attention) on 8 TRN2 NeuronCores.

Strategy: data-parallel over batch (4 sequences per core), embedding table /
pos table / projection weights replicated to every core.

Per-core pipeline (per sequence):
  gather word_emb rows (indirect DMA) -> += pos_emb (DVE)
  LayerNorm: bn_stats/bn_aggr (DVE), rsqrt as exp(-.5*ln(var+eps)) (ACT),
             normalize via ACT Identity(scale=rs, bias=-mu*rs) -> x (fp32r)
  x^T via PE transpose (fp32r)
  Q^T = Wq'-slices @ x^T, K^T likewise (fp32r, gamma and 1/sqrt(D) folded into
  Wq' on host); V = x^T-slices @ Wv' (stored fp16)
  S = Q^T.T @ K^T in PSUM; row-max (DVE, negated); P = exp(S - max) on ACT
  (fp16) with accumulated row sums l; P^T via DMA transpose (fp16)
  O = P@V accumulated in PSUM; final scale by 1/l (DVE) -> DMA out
"""
import math
import numpy as np

from concourse import bass, mybir
import concourse.tile as tile
from concourse.bass_utils import run_bass_kernel_spmd
from concourse.masks import make_identity

P = 128
D = 512
VOC = 32000
N_CORES = 8

F32 = mybir.dt.float32
F32R = mybir.dt.float32r
F16 = mybir.dt.float16
I32 = mybir.dt.int32

AF = mybir.ActivationFunctionType
ALU = mybir.AluOpType
AX = mybir.AxisListType


def fix_fat_waits(nc, max_waits=1):
    """Walrus rejects instructions carrying more than ~1 semaphore wait. Tile
    occasionally emits joins (notably the kernel-tail drain) with one wait per
    producing processor. Split the extras into a chain of single-wait NoOps on
    the same engine, inserted immediately before the original instruction."""
    n_new = 0
    for bb in nc.main_func.blocks:
        insts = bb.instructions
        i = 0
        while i < len(insts):
            ins = insts[i]
            si = ins.sync_info
            if si and si.on_wait and len(si.on_wait) > max_waits:
                waits = list(si.on_wait)
                keep = waits[-max_waits:]
                extra = waits[:-max_waits]
                ins.sync_info = mybir.SyncInfo(
                    on_wait=keep, on_update=list(si.on_update or []))
                for j, w in enumerate(extra):
                    nop = mybir.InstNoOp(name=f"W-split-{n_new}", ins=[], outs=[])
                    n_new += 1
                    nop.engine = ins.engine
                    nop.sync_info = mybir.SyncInfo(on_wait=[w], on_update=[])
                    insts.insert(i + j, nop)
                i += len(extra)
            i += 1
    return n_new


def build(b_per_core: int, s_len: int, voc: int, apply_beta: bool, debug: bool = False):
    """Build the per-core SPMD program. All cores run this same module."""
    nt = s_len // P          # token tiles per sequence (8)
    dk = D // P              # feature tiles (4)
    nchunk = 2               # moving-dim chunks for N=s_len matmuls (512 each)
    ckw = s_len // nchunk    # 512

    nc = bass.Bass()

    dbg = {}
    if debug:
        dbg["x"] = nc.dram_tensor("dbg_x", [P, nt * D], F32, kind="ExternalOutput")
        dbg["xt"] = nc.dram_tensor("dbg_xt", [P, dk * s_len], F32, kind="ExternalOutput")
        dbg["qt"] = nc.dram_tensor("dbg_qt", [P, dk * s_len], F32, kind="ExternalOutput")
        dbg["kt"] = nc.dram_tensor("dbg_kt", [P, dk * s_len], F32, kind="ExternalOutput")
        dbg["v"] = nc.dram_tensor("dbg_v", [P, nt * D], F32, kind="ExternalOutput")
        dbg["s"] = nc.dram_tensor("dbg_s", [P, s_len], F32, kind="ExternalOutput")
        dbg["p"] = nc.dram_tensor("dbg_p", [P, s_len], F32, kind="ExternalOutput")
        dbg["pt"] = nc.dram_tensor("dbg_pt", [P, nt * P], F32, kind="ExternalOutput")

    ids_d = nc.dram_tensor("ids", [P, b_per_core * nt], I32, kind="ExternalInput")
    wemb_d = nc.dram_tensor("wemb", [voc, D], F32, kind="ExternalInput")
    pos_d = nc.dram_tensor("pos", [s_len, D], F32, kind="ExternalInput")
    wq_d = nc.dram_tensor("wq", [D, D], F32, kind="ExternalInput")
    wk_d = nc.dram_tensor("wk", [D, D], F32, kind="ExternalInput")
    wv_d = nc.dram_tensor("wv", [D, D], F32, kind="ExternalInput")
    if apply_beta:
        beta_d = nc.dram_tensor("beta_b", [P, D], F32, kind="ExternalInput")
    out_d = nc.dram_tensor("out", [b_per_core * s_len, D], F32, kind="ExternalOutput")

    with tile.TileContext(nc) as tc:
        with (
            tc.tile_pool(name="pers", bufs=1) as pers,
            tc.tile_pool(name="ebuf", bufs=2) as ebuf,
            tc.tile_pool(name="proj", bufs=1) as proj,
            tc.tile_pool(name="sm", bufs=2) as sm,
            tc.tile_pool(name="ps_misc", bufs=2, space="PSUM") as ps_misc,
            tc.tile_pool(name="ps_s", bufs=2, space="PSUM") as ps_s,
        ):
            # ---------------- preamble: persistent tiles ----------------
            ids_t = pers.tile([P, b_per_core * nt], I32, tag="ids")
            nc.sync.dma_start(out=ids_t[:], in_=ids_d[:, :])

            pos_t = pers.tile([P, nt, D], F32, tag="pos")
            nc.sync.dma_start(
                out=pos_t[:],
                in_=pos_d.rearrange("(a p) d -> p a d", p=P),
            )

            w_stage = pers.tile([P, dk, D], F32, tag="wstage")
            w_r = {}
            for name, wd in (("wq", wq_d), ("wk", wk_d), ("wv", wv_d)):
                wt = pers.tile([P, dk, D], F32R, tag=f"{name}r")
                nc.sync.dma_start(
                    out=w_stage[:], in_=wd.rearrange("(a p) n -> p a n", p=P))
                nc.vector.tensor_copy(out=wt[:], in_=w_stage[:])
                w_r[name] = wt

            if apply_beta:
                beta_t = pers.tile([P, D], F32, tag="betab")
                nc.sync.dma_start(out=beta_t[:], in_=beta_d[:, :])

            ident_f = pers.tile([P, P], F32, tag="ident_f")
            make_identity(nc, ident_f[:])
            ident = pers.tile([P, P], F32R, tag="ident")
            nc.vector.tensor_copy(out=ident[:], in_=ident_f[:])

            epsb = pers.tile([P, 1], F32, tag="epsb")
            nc.gpsimd.memset(epsb[:], 1e-5)

            # ---------------- per-sequence pipeline ----------------
            for b in range(b_per_core):
                # --- embed + layernorm ---
                e_all = ebuf.tile([P, nt, D], F32, tag="e_all")
                for j in range(nt):
                    nc.gpsimd.indirect_dma_start(
                        out=e_all[:, j, :],
                        out_offset=None,
                        in_=wemb_d[:],
                        in_offset=bass.IndirectOffsetOnAxis(
                            ap=ids_t[:, b * nt + j: b * nt + j + 1], axis=0),
                    )
                for j in range(nt):
                    nc.vector.tensor_tensor(
                        out=e_all[:, j, :], in0=e_all[:, j, :],
                        in1=pos_t[:, j, :], op=ALU.add)

                s6 = ebuf.tile([P, nt, 6], F32, tag="s6")
                mv = ebuf.tile([P, nt, 2], F32, tag="mv")
                for j in range(nt):
                    nc.vector.bn_stats(out=s6[:, j, :], in_=e_all[:, j, :])
                    nc.vector.bn_aggr(out=mv[:, j, :], in_=s6[:, j, :])

                lnv = ebuf.tile([P, nt], F32, tag="lnv")
                rs = ebuf.tile([P, nt], F32, tag="rs")
                nmurs = ebuf.tile([P, nt], F32, tag="nmurs")
                # rs = exp(-0.5*ln(var + eps)) == rsqrt(var + eps)
                nc.scalar.activation(out=lnv[:], in_=mv[:, :, 1], func=AF.Ln,
                                     bias=epsb[:, 0:1], scale=1.0)
                nc.scalar.activation(out=rs[:], in_=lnv[:], func=AF.Exp,
                                     bias=0.0, scale=-0.5)
                # nmurs = -mean * rs
                nc.vector.tensor_tensor(out=nmurs[:], in0=mv[:, :, 0],
                                        in1=rs[:], op=ALU.mult)
                nc.vector.tensor_scalar_mul(out=nmurs[:], in0=nmurs[:],
                                            scalar1=-1.0)

                x_all = ebuf.tile([P, nt, D], F32R, tag="x_all")
                for j in range(nt):
                    if apply_beta:
                        xtmp = ebuf.tile([P, D], F32, tag="xtmp")
                        nc.scalar.activation(
                            out=xtmp[:], in_=e_all[:, j, :], func=AF.Identity,
                            bias=nmurs[:, j:j + 1], scale=rs[:, j:j + 1])
                        nc.vector.tensor_tensor(
                            out=x_all[:, j, :], in0=xtmp[:], in1=beta_t[:],
                            op=ALU.add)
                    else:
                        nc.scalar.activation(
                            out=x_all[:, j, :], in_=e_all[:, j, :],
                            func=AF.Identity,
                            bias=nmurs[:, j:j + 1], scale=rs[:, j:j + 1])

                if debug and b == 0:
                    nc.gpsimd.dma_start(
                        out=dbg["x"][:, :],
                        in_=x_all[:].bitcast(F32).rearrange("p a d -> p (a d)"))

                # --- x^T via PE transpose: xT[p, c, j*128+q] = x[q, j, c*128+p]
                xt = ebuf.tile([P, dk, s_len], F32R, tag="xt")
                for j in range(nt):
                    pst = ps_misc.tile([P, dk, P], F32R, tag="ps_misc")
                    for c in range(dk):
                        nc.tensor.transpose(
                            out=pst[:, c, :],
                            in_=x_all[:, j, c * P:(c + 1) * P],
                            identity=ident[:])
                    nc.vector.tensor_copy(
                        out=xt[:, :, j * P:(j + 1) * P], in_=pst[:])

                if debug and b == 0:
                    nc.gpsimd.dma_start(
                        out=dbg["xt"][:, :],
                        in_=xt[:].bitcast(F32).rearrange("p a d -> p (a d)"))

                # --- projections ---
                qt = proj.tile([P, dk, s_len], F32R, tag="qt")
                kt = proj.tile([P, dk, s_len], F32R, tag="kt")
                for wname, dst in (("wq", qt), ("wk", kt)):
                    wt = w_r[wname]
                    for dj in range(dk):
                        for ch in range(nchunk):
                            ps = ps_misc.tile([P, ckw], F32, tag="ps_misc")
                            for di in range(dk):
                                nc.tensor.matmul(
                                    out=ps[:],
                                    lhsT=wt[:, di, dj * P:(dj + 1) * P],
                                    rhs=xt[:, di, ch * ckw:(ch + 1) * ckw],
                                    start=(di == 0), stop=(di == dk - 1))
                            nc.scalar.copy(
                                out=dst[:, dj, ch * ckw:(ch + 1) * ckw],
                                in_=ps[:])

                v16 = proj.tile([P, nt, D], F16, tag="v16")
                for j in range(nt):
                    ps = ps_misc.tile([P, D], F32, tag="ps_misc")
                    for di in range(dk):
                        nc.tensor.matmul(
                            out=ps[:],
                            lhsT=xt[:, di, j * P:(j + 1) * P],
                            rhs=w_r["wv"][:, di, :],
                            start=(di == 0), stop=(di == dk - 1))
                    nc.scalar.copy(out=v16[:, j, :], in_=ps[:])

                if debug and b == 0:
                    nc.gpsimd.dma_start(
                        out=dbg["qt"][:, :],
                        in_=qt[:].bitcast(F32).rearrange("p a d -> p (a d)"))
                    nc.gpsimd.dma_start(
                        out=dbg["kt"][:, :],
                        in_=kt[:].bitcast(F32).rearrange("p a d -> p (a d)"))
                    vf = ebuf.tile([P, nt, D], F32, tag="dbg_vf")
                    nc.vector.tensor_copy(out=vf[:], in_=v16[:])
                    nc.gpsimd.dma_start(
                        out=dbg["v"][:, :],
                        in_=vf[:].rearrange("p a d -> p (a d)"))

                # --- attention per q-tile ---
                nm = ebuf.tile([P, nt], F32, tag="nm")
                ls = ebuf.tile([P, nt], F32, tag="ls")
                rr = ebuf.tile([P, nt], F32, tag="rr")
                for j in range(nt):
                    # one full PSUM bank (512 f32) per chunk: start=True
                    # clears the whole bank, so accumulation groups must not
                    # share banks.
                    s_ps = ps_s.tile([P, nchunk, 512], F32, tag="s_ps")
                    for ch in range(nchunk):
                        for h in range(dk):
                            nc.tensor.matmul(
                                out=s_ps[:, ch, :ckw],
                                lhsT=qt[:, h, j * P:(j + 1) * P],
                                rhs=kt[:, h, ch * ckw:(ch + 1) * ckw],
                                start=(h == 0), stop=(h == dk - 1))
                    if debug and b == 0 and j == 0:
                        sf = ebuf.tile([P, s_len], F32, tag="dbg_sf")
                        nc.vector.tensor_copy(
                            out=sf[:].rearrange("p (a d) -> p a d", a=nchunk),
                            in_=s_ps[:, :, :ckw])
                        nc.gpsimd.dma_start(out=dbg["s"][:, :], in_=sf[:])
                    nc.vector.tensor_reduce(
                        out=nm[:, j:j + 1], in_=s_ps[:, :, :ckw], axis=AX.XY,
                        op=ALU.max, negate=True)
                    p16 = sm.tile([P, s_len], F16, tag="p16")
                    nc.scalar.activation(
                        out=p16[:].rearrange("p (a d) -> p a d", a=nchunk),
                        in_=s_ps[:, :, :ckw], func=AF.Exp,
                        bias=nm[:, j:j + 1], scale=1.0,
                        accum_out=ls[:, j:j + 1])
                    nc.vector.reciprocal(out=rr[:, j:j + 1], in_=ls[:, j:j + 1])
                    pt16 = sm.tile([P, nt, P], F16, tag="pt16")
                    nc.sync.dma_start_transpose(pt16[:], p16[:])

                    if debug and b == 0 and j == 0:
                        pf = ebuf.tile([P, s_len], F32, tag="dbg_pf")
                        nc.vector.tensor_copy(out=pf[:], in_=p16[:])
                        nc.gpsimd.dma_start(out=dbg["p"][:, :], in_=pf[:])
                        ptf = ebuf.tile([P, nt * P], F32, tag="dbg_ptf")
                        nc.vector.tensor_copy(
                            out=ptf[:], in_=pt16[:].rearrange("p a d -> p (a d)"))
                        nc.gpsimd.dma_start(out=dbg["pt"][:, :], in_=ptf[:])

                    o_ps = ps_misc.tile([P, D], F32, tag="ps_misc")
                    for k in range(nt):
                        nc.tensor.matmul(
                            out=o_ps[:],
                            lhsT=pt16[:, k, :],
                            rhs=v16[:, k, :],
                            start=(k == 0), stop=(k == nt - 1))
                    o_sb = sm.tile([P, D], F32, tag="o_sb")
                    nc.vector.tensor_scalar_mul(
                        out=o_sb[:], in0=o_ps[:],
                        scalar1=rr[:, j:j + 1])
                    row = (b * nt + j) * P
                    nc.gpsimd.dma_start(
                        out=out_d[row:row + P, :], in_=o_sb[:])

    fix_fat_waits(nc)
    return nc


_CACHE = {}


def _get_module(b_per_core, s_len, voc, apply_beta):
    key = (b_per_core, s_len, voc, apply_beta)
    if key not in _CACHE:
        _CACHE[key] = build(*key)
    return _CACHE[key]


def kernel(input, word_emb, pos_emb, gamma, beta, Wk, Wq, Wv):
    input = np.asarray(input)
    word_emb = np.ascontiguousarray(np.asarray(word_emb, dtype=np.float32))
    pos_emb = np.asarray(pos_emb, dtype=np.float32)
    gamma = np.asarray(gamma, dtype=np.float32)
    beta = np.asarray(beta, dtype=np.float32)
    Wk = np.asarray(Wk, dtype=np.float32)
    Wq = np.asarray(Wq, dtype=np.float32)
    Wv = np.asarray(Wv, dtype=np.float32)

    B, S = input.shape
    voc, d = word_emb.shape
    assert d == D
    b_per_core = B // N_CORES
    nt = S // P

    # fold gamma (scales x along d) and 1/sqrt(D) into the projection weights
    g64 = gamma.astype(np.float64)
    wq_s = (Wq.astype(np.float64) * g64[:, None] / math.sqrt(D)).astype(np.float32)
    wk_s = (Wk.astype(np.float64) * g64[:, None]).astype(np.float32)
    wv_s = (Wv.astype(np.float64) * g64[:, None]).astype(np.float32)

    apply_beta = bool(np.any(beta != 0.0))
    pos_c = np.ascontiguousarray(pos_emb[:S])

    nc = _get_module(b_per_core, S, voc, apply_beta)

    ids32 = input.astype(np.int32)  # [B, S]
    in_maps = []
    for c in range(N_CORES):
        shard = ids32[c * b_per_core:(c + 1) * b_per_core]       # [bpc, S]
        ids_col = np.ascontiguousarray(
            shard.reshape(b_per_core * nt, P).T)                 # [128, bpc*nt]
        m = {
            "ids": ids_col,
            "wemb": word_emb,
            "pos": pos_c,
            "wq": wq_s,
            "wk": wk_s,
            "wv": wv_s,
        }
        if apply_beta:
            m["beta_b"] = np.ascontiguousarray(
                np.broadcast_to(beta, (P, D)).astype(np.float32))
        in_maps.append(m)

    res = run_bass_kernel_spmd(nc, in_maps, core_ids=list(range(N_CORES)))
    out = np.concatenate(
        [r["out"].reshape(b_per_core, S, D) for r in res.results], axis=0)
    return out


# revision 21
# speedup vs baseline: 308.5360x; 308.5360x over previous
"""MiniBert (embed + LayerNorm + single-head attention) on 8 TRN2 NeuronCores.

Strategy: data-parallel over batch (4 sequences per core), embedding table /
pos table / projection weights replicated to every core.

Per-core pipeline (per sequence):
  gather word_emb rows (indirect DMA) -> += pos_emb (DVE)
  LayerNorm: bn_stats/bn_aggr (DVE), rsqrt as exp(-.5*ln(var+eps)) (ACT),
             normalize via ACT Identity(scale=rs, bias=-mu*rs) -> x (fp32r)
  x^T via PE transpose (fp32r)
  Q^T = Wq'-slices @ x^T, K^T likewise (fp32r, gamma and 1/sqrt(D) folded into
  Wq' on host); V = x^T-slices @ Wv' (stored fp16)
  S = Q^T.T @ K^T in PSUM; row-max (DVE, negated); P = exp(S - max) on ACT
  (fp16) with accumulated row sums l; P^T via DMA transpose (fp16)
  O = P@V accumulated in PSUM; final scale by 1/l (DVE) -> DMA out
"""
import math
import numpy as np

from concourse import bass, mybir
import concourse.tile as tile
from concourse.bass_utils import run_bass_kernel_spmd
from concourse.masks import make_identity

P = 128
D = 512
VOC = 32000
N_CORES = 8

F32 = mybir.dt.float32
F32R = mybir.dt.float32r
F16 = mybir.dt.float16
I32 = mybir.dt.int32

AF = mybir.ActivationFunctionType
ALU = mybir.AluOpType
AX = mybir.AxisListType


def fix_fat_waits(nc, max_waits=1):
    """Walrus rejects instructions carrying more than ~1 semaphore wait. Tile
    occasionally emits joins (notably the kernel-tail drain) with one wait per
    producing processor. Split the extras into a chain of single-wait NoOps on
    the same engine, inserted immediately before the original instruction."""
    n_new = 0
    for bb in nc.main_func.blocks:
        insts = bb.instructions
        i = 0
        while i < len(insts):
            ins = insts[i]
            si = ins.sync_info
            if si and si.on_wait and len(si.on_wait) > max_waits:
                waits = list(si.on_wait)
                keep = waits[-max_waits:]
                extra = waits[:-max_waits]
                ins.sync_info = mybir.SyncInfo(
                    on_wait=keep, on_update=list(si.on_update or []))
                for j, w in enumerate(extra):
                    nop = mybir.InstNoOp(name=f"W-split-{n_new}", ins=[], outs=[])
                    n_new += 1
                    nop.engine = ins.engine
                    nop.sync_info = mybir.SyncInfo(on_wait=[w], on_update=[])
                    insts.insert(i + j, nop)
                i += len(extra)
            i += 1
    return n_new


def build(b_per_core: int, s_len: int, voc: int, apply_beta: bool, debug: bool = False, stages: int = 99,
          bufs_ebuf: int = 2, bufs_sm: int = 3, bufs_psm: int = 2, bufs_pss: int = 3):
    """Build the per-core SPMD program. All cores run this same module."""
    nt = s_len // P          # token tiles per sequence (8)
    dk = D // P              # feature tiles (4)
    nchunk = 2               # moving-dim chunks for N=s_len matmuls (512 each)
    ckw = s_len // nchunk    # 512

    nc = bass.Bass()

    dbg = {}
    if debug:
        dbg["x"] = nc.dram_tensor("dbg_x", [P, nt * D], F32, kind="ExternalOutput")
        dbg["xt"] = nc.dram_tensor("dbg_xt", [P, dk * s_len], F32, kind="ExternalOutput")
        dbg["qt"] = nc.dram_tensor("dbg_qt", [P, dk * s_len], F32, kind="ExternalOutput")
        dbg["kt"] = nc.dram_tensor("dbg_kt", [P, dk * s_len], F32, kind="ExternalOutput")
        dbg["v"] = nc.dram_tensor("dbg_v", [P, nt * D], F32, kind="ExternalOutput")
        dbg["s"] = nc.dram_tensor("dbg_s", [P, s_len], F32, kind="ExternalOutput")
        dbg["p"] = nc.dram_tensor("dbg_p", [P, s_len], F32, kind="ExternalOutput")
        dbg["pt"] = nc.dram_tensor("dbg_pt", [P, nt * P], F32, kind="ExternalOutput")

    ids_d = nc.dram_tensor("ids", [P, b_per_core * nt], I32, kind="ExternalInput")
    wemb_d = nc.dram_tensor("wemb", [voc, D], F32, kind="ExternalInput")
    pos_d = nc.dram_tensor("pos", [s_len, D], F32, kind="ExternalInput")
    wq_d = nc.dram_tensor("wq", [D, D], F32, kind="ExternalInput")
    wk_d = nc.dram_tensor("wk", [D, D], F32, kind="ExternalInput")
    wv_d = nc.dram_tensor("wv", [D, D], F32, kind="ExternalInput")
    if apply_beta:
        beta_d = nc.dram_tensor("beta_b", [P, D], F32, kind="ExternalInput")
    out_d = nc.dram_tensor("out", [b_per_core * s_len, D], F32, kind="ExternalOutput")

    with tile.TileContext(nc) as tc:
        with (
            tc.tile_pool(name="pers", bufs=1) as pers,
            tc.tile_pool(name="ebuf", bufs=bufs_ebuf) as ebuf,
            tc.tile_pool(name="proj", bufs=1) as proj,
            tc.tile_pool(name="sm", bufs=bufs_sm) as sm,
            tc.tile_pool(name="ps_misc", bufs=bufs_psm, space="PSUM") as ps_misc,
            tc.tile_pool(name="ps_s", bufs=bufs_pss, space="PSUM") as ps_s,
        ):
            # ---------------- preamble: persistent tiles ----------------
            ids_t = pers.tile([P, b_per_core * nt], I32, tag="ids")
            nc.sync.dma_start(out=ids_t[:], in_=ids_d[:, :])

            pos_t = pers.tile([P, nt, D], F32, tag="pos")
            nc.sync.dma_start(
                out=pos_t[:],
                in_=pos_d.rearrange("(a p) d -> p a d", p=P),
            )

            w_stage = pers.tile([P, dk, D], F32, tag="wstage")
            w_r = {}
            for name, wd in (("wq", wq_d), ("wk", wk_d), ("wv", wv_d)):
                wt = pers.tile([P, dk, D], F32R, tag=f"{name}r")
                nc.sync.dma_start(
                    out=w_stage[:], in_=wd.rearrange("(a p) n -> p a n", p=P))
                nc.vector.tensor_copy(out=wt[:], in_=w_stage[:])
                w_r[name] = wt

            if apply_beta:
                beta_t = pers.tile([P, D], F32, tag="betab")
                nc.sync.dma_start(out=beta_t[:], in_=beta_d[:, :])

            ident_f = pers.tile([P, P], F32, tag="ident_f")
            make_identity(nc, ident_f[:])
            ident = pers.tile([P, P], F32R, tag="ident")
            nc.vector.tensor_copy(out=ident[:], in_=ident_f[:])

            epsb = pers.tile([P, 1], F32, tag="epsb")
            nc.gpsimd.memset(epsb[:], 1e-5)

            # ---------------- per-sequence pipeline ----------------
            for b in range(b_per_core):
                if stages_eff < 1:
                    continue
                # --- embed + layernorm ---
                e_all = ebuf.tile([P, nt, D], F32, tag="e_all")
                for j in range(nt):
                    nc.gpsimd.indirect_dma_start(
                        out=e_all[:, j, :],
                        out_offset=None,
                        in_=wemb_d[:],
                        in_offset=bass.IndirectOffsetOnAxis(
                            ap=ids_t[:, b * nt + j: b * nt + j + 1], axis=0),
                    )
                if stages_eff < 2:
                    continue
                for j in range(nt):
                    nc.vector.tensor_tensor(
                        out=e_all[:, j, :], in0=e_all[:, j, :],
                        in1=pos_t[:, j, :], op=ALU.add)

                s6 = ebuf.tile([P, nt, 6], F32, tag="s6")
                mv = ebuf.tile([P, nt, 2], F32, tag="mv")
                for j in range(nt):
                    nc.vector.bn_stats(out=s6[:, j, :], in_=e_all[:, j, :])
                    nc.vector.bn_aggr(out=mv[:, j, :], in_=s6[:, j, :])

                lnv = ebuf.tile([P, nt], F32, tag="lnv")
                rs = ebuf.tile([P, nt], F32, tag="rs")
                nmurs = ebuf.tile([P, nt], F32, tag="nmurs")
                # rs = exp(-0.5*ln(var + eps)) == rsqrt(var + eps)
                nc.scalar.activation(out=lnv[:], in_=mv[:, :, 1], func=AF.Ln,
                                     bias=epsb[:, 0:1], scale=1.0)
                nc.scalar.activation(out=rs[:], in_=lnv[:], func=AF.Exp,
                                     bias=0.0, scale=-0.5)
                # nmurs = -mean * rs
                nc.vector.tensor_tensor(out=nmurs[:], in0=mv[:, :, 0],
                                        in1=rs[:], op=ALU.mult)
                nc.vector.tensor_scalar_mul(out=nmurs[:], in0=nmurs[:],
                                            scalar1=-1.0)

                x_all = ebuf.tile([P, nt, D], F32R, tag="x_all")
                for j in range(nt):
                    if apply_beta:
                        xtmp = ebuf.tile([P, D], F32, tag="xtmp")
                        nc.scalar.activation(
                            out=xtmp[:], in_=e_all[:, j, :], func=AF.Identity,
                            bias=nmurs[:, j:j + 1], scale=rs[:, j:j + 1])
                        nc.vector.tensor_tensor(
                            out=x_all[:, j, :], in0=xtmp[:], in1=beta_t[:],
                            op=ALU.add)
                    else:
                        nc.scalar.activation(
                            out=x_all[:, j, :], in_=e_all[:, j, :],
                            func=AF.Identity,
                            bias=nmurs[:, j:j + 1], scale=rs[:, j:j + 1])

                if debug and b == 0:
                    nc.gpsimd.dma_start(
                        out=dbg["x"][:, :],
                        in_=x_all[:].bitcast(F32).rearrange("p a d -> p (a d)"))

                if stages_eff < 3:
                    continue
                # --- x^T via PE transpose: xT[p, c, j*128+q] = x[q, j, c*128+p]
                xt = ebuf.tile([P, dk, s_len], F32R, tag="xt")
                for j in range(nt):
                    pst = ps_misc.tile([P, dk, P], F32R, tag="ps_misc")
                    for c in range(dk):
                        nc.tensor.transpose(
                            out=pst[:, c, :],
                            in_=x_all[:, j, c * P:(c + 1) * P],
                            identity=ident[:])
                    nc.vector.tensor_copy(
                        out=xt[:, :, j * P:(j + 1) * P], in_=pst[:])

                if debug and b == 0:
                    nc.gpsimd.dma_start(
                        out=dbg["xt"][:, :],
                        in_=xt[:].bitcast(F32).rearrange("p a d -> p (a d)"))

                if stages_eff < 4:
                    continue
                # --- projections ---
                qt = proj.tile([P, dk, s_len], F32R, tag="qt")
                kt = proj.tile([P, dk, s_len], F32R, tag="kt")
                for wname, dst in (("wq", qt), ("wk", kt)):
                    wt = w_r[wname]
                    for dj in range(dk):
                        for ch in range(nchunk):
                            ps = ps_misc.tile([P, ckw], F32, tag="ps_misc")
                            for di in range(dk):
                                nc.tensor.matmul(
                                    out=ps[:],
                                    lhsT=wt[:, di, dj * P:(dj + 1) * P],
                                    rhs=xt[:, di, ch * ckw:(ch + 1) * ckw],
                                    start=(di == 0), stop=(di == dk - 1))
                            nc.scalar.copy(
                                out=dst[:, dj, ch * ckw:(ch + 1) * ckw],
                                in_=ps[:])

                v16 = proj.tile([P, nt, D], F16, tag="v16")
                for j in range(nt):
                    ps = ps_misc.tile([P, D], F32, tag="ps_misc")
                    for di in range(dk):
                        nc.tensor.matmul(
                            out=ps[:],
                            lhsT=xt[:, di, j * P:(j + 1) * P],
                            rhs=w_r["wv"][:, di, :],
                            start=(di == 0), stop=(di == dk - 1))
                    nc.scalar.copy(out=v16[:, j, :], in_=ps[:])

                if debug and b == 0:
                    nc.gpsimd.dma_start(
                        out=dbg["qt"][:, :],
                        in_=qt[:].bitcast(F32).rearrange("p a d -> p (a d)"))
                    nc.gpsimd.dma_start(
                        out=dbg["kt"][:, :],
                        in_=kt[:].bitcast(F32).rearrange("p a d -> p (a d)"))
                    vf = ebuf.tile([P, nt, D], F32, tag="dbg_vf")
                    nc.vector.tensor_copy(out=vf[:], in_=v16[:])
                    nc.gpsimd.dma_start(
                        out=dbg["v"][:, :],
                        in_=vf[:].rearrange("p a d -> p (a d)"))

                if stages_eff < 5:
                    continue
                # --- attention per q-tile ---
                nm = ebuf.tile([P, nt], F32, tag="nm")
                ls = ebuf.tile([P, nt], F32, tag="ls")
                rr = ebuf.tile([P, nt], F32, tag="rr")
                for j in range(nt):
                    # one full PSUM bank (512 f32) per chunk: start=True
                    # clears the whole bank, so accumulation groups must not
                    # share banks.
                    s_ps = ps_s.tile([P, nchunk, 512], F32, tag="s_ps")
                    for ch in range(nchunk):
                        for h in range(dk):
                            nc.tensor.matmul(
                                out=s_ps[:, ch, :ckw],
                                lhsT=qt[:, h, j * P:(j + 1) * P],
                                rhs=kt[:, h, ch * ckw:(ch + 1) * ckw],
                                start=(h == 0), stop=(h == dk - 1))
                    if debug and b == 0 and j == 0:
                        sf = ebuf.tile([P, s_len], F32, tag="dbg_sf")
                        nc.vector.tensor_copy(
                            out=sf[:].rearrange("p (a d) -> p a d", a=nchunk),
                            in_=s_ps[:, :, :ckw])
                        nc.gpsimd.dma_start(out=dbg["s"][:, :], in_=sf[:])
                    nc.vector.tensor_reduce(
                        out=nm[:, j:j + 1], in_=s_ps[:, :, :ckw], axis=AX.XY,
                        op=ALU.max, negate=True)
                    p16 = sm.tile([P, s_len], F16, tag="p16")
                    nc.scalar.activation(
                        out=p16[:].rearrange("p (a d) -> p a d", a=nchunk),
                        in_=s_ps[:, :, :ckw], func=AF.Exp,
                        bias=nm[:, j:j + 1], scale=1.0,
                        accum_out=ls[:, j:j + 1])
                    nc.vector.reciprocal(out=rr[:, j:j + 1], in_=ls[:, j:j + 1])
                    pt16 = sm.tile([P, nt, P], F16, tag="pt16")
                    nc.sync.dma_start_transpose(pt16[:], p16[:])

                    if debug and b == 0 and j == 0:
                        pf = ebuf.tile([P, s_len], F32, tag="dbg_pf")
                        nc.vector.tensor_copy(out=pf[:], in_=p16[:])
                        nc.gpsimd.dma_start(out=dbg["p"][:, :], in_=pf[:])
                        ptf = ebuf.tile([P, nt * P], F32, tag="dbg_ptf")
                        nc.vector.tensor_copy(
                            out=ptf[:], in_=pt16[:].rearrange("p a d -> p (a d)"))
                        nc.gpsimd.dma_start(out=dbg["pt"][:, :], in_=ptf[:])

                    if stages_eff < 6:
                        continue
                    o_ps = ps_misc.tile([P, D], F32, tag="ps_misc")
                    for k in range(nt):
                        nc.tensor.matmul(
                            out=o_ps[:],
                            lhsT=pt16[:, k, :],
                            rhs=v16[:, k, :],
                            start=(k == 0), stop=(k == nt - 1))
                    o_sb = sm.tile([P, D], F32, tag="o_sb")
                    nc.vector.tensor_scalar_mul(
                        out=o_sb[:], in0=o_ps[:],
                        scalar1=rr[:, j:j + 1])
                    row = (b * nt + j) * P
                    nc.gpsimd.dma_start(
                        out=out_d[row:row + P, :], in_=o_sb[:])

    fix_fat_waits(nc)
    return nc


_CACHE = {}


def _get_module(b_per_core, s_len, voc, apply_beta, stages=99):
    key = (b_per_core, s_len, voc, apply_beta, stages)
    if key not in _CACHE:
        _CACHE[key] = build(b_per_core, s_len, voc, apply_beta, stages=stages)
    return _CACHE[key]


def kernel(input, word_emb, pos_emb, gamma, beta, Wk, Wq, Wv):
    input = np.asarray(input)
    word_emb = np.ascontiguousarray(np.asarray(word_emb, dtype=np.float32))
    pos_emb = np.asarray(pos_emb, dtype=np.float32)
    gamma = np.asarray(gamma, dtype=np.float32)
    beta = np.asarray(beta, dtype=np.float32)
    Wk = np.asarray(Wk, dtype=np.float32)
    Wq = np.asarray(Wq, dtype=np.float32)
    Wv = np.asarray(Wv, dtype=np.float32)

    B, S = input.shape
    voc, d = word_emb.shape
    assert d == D
    b_per_core = B // N_CORES
    nt = S // P

    # fold gamma (scales x along d) and 1/sqrt(D) into the projection weights
    g64 = gamma.astype(np.float64)
    wq_s = (Wq.astype(np.float64) * g64[:, None] / math.sqrt(D)).astype(np.float32)
    wk_s = (Wk.astype(np.float64) * g64[:, None]).astype(np.float32)
    wv_s = (Wv.astype(np.float64) * g64[:, None]).astype(np.float32)

    apply_beta = bool(np.any(beta != 0.0))
    pos_c = np.ascontiguousarray(pos_emb[:S])

    nc = _get_module(b_per_core, S, voc, apply_beta)

    ids32 = input.astype(np.int32)  # [B, S]
    in_maps = []
    for c in range(N_CORES):
        shard = ids32[c * b_per_core:(c + 1) * b_per_core]       # [bpc, S]
        ids_col = np.ascontiguousarray(
            shard.reshape(b_per_core * nt, P).T)                 # [128, bpc*nt]
        m = {
            "ids": ids_col,
            "wemb": word_emb,
            "pos": pos_c,
            "wq": wq_s,
            "wk": wk_s,
            "wv": wv_s,
        }
        if apply_beta:
            # gamma is folded into the projection weights, so the device
            # kernel computes (xhat + b) @ (gamma*W). Feeding b = beta/gamma
            # makes that equal xhat@(gamma*W) + beta@W, the reference value.
            beta_eff = (beta.astype(np.float64)
                        / np.where(g64 == 0.0, 1.0, g64)).astype(np.float32)
            m["beta_b"] = np.ascontiguousarray(
                np.broadcast_to(beta_eff, (P, D)).astype(np.float32))
        in_maps.append(m)

    res = run_bass_kernel_spmd(nc, in_maps, core_ids=list(range(N_CORES)))
    out = np.concatenate(
        [r["out"].reshape(b_per_core, S, D) for r in res.results], axis=0)
    return out
